# revision 12
# baseline (speedup 1.0000x reference)
"""Trainium2 Bass kernel for nn_ClusterCritic (gnn_message_passing).

Data-parallel over 8 NeuronCores: batch axis B=32768 sharded 8x4096.
Per core, two phases:
  phase 1: stream inputs (fp32->bf16 cast DMA), PE-transpose to feature-major
           x^T tiles resident in SBUF, accumulate per-feature sum / sum-of-
           squares in PSUM via PE matmuls (batch is the contraction axis).
  barrier: AllReduce of the (sum, sumsq) stats across the 8 cores, then fold
           BatchNorm into the encoder weights (W' = W*rsqrt(var+eps),
           b' = b - m @ W'^T) on device.
  phase 2: per 512-batch chunk: fused sa/s encoder matmuls -> bias+leaky-relu
           -> fused keys/vals/sel matmul emitted directly in batch-major
           (activation as the stationary operand) -> leave-one-out attention
           (products + d/j tree-contractions on DVE, exp on ACT) -> outputs.
"""

import numpy as np
import ml_dtypes

# ---- static problem configuration (hardcoded per contract) ----
A, C, PER = 10, 5, 2
B = 32768
SD, AD = 100, 20
F = SD + AD            # 120 features per agent
HID, H, D = 32, 4, 8
ND = H * D             # 32
NCORES = 8
BC = B // NCORES       # 4096 batch per core
NCH = 8                # chunks per core
BCH = BC // NCH        # 512 batch per chunk
NSUB = 4               # 128-row subtiles per chunk
BN_EPS = 1e-5
INV_SQRT_D = float(1.0 / np.sqrt(D))

_BUILT = None  # cached Bass module
TRACE = False
LAST_RESULTS = None


def _build_bass(single=False):
    import contextlib
    import concourse.bass as bass
    import concourse.tile as tile
    from concourse import bacc, mybir

    f32 = mybir.dt.float32
    bf16 = mybir.dt.bfloat16
    AluT = mybir.AluOpType
    AF = mybir.ActivationFunctionType

    ncores = 1 if single else NCORES
    nc = bacc.Bacc("TRN2", target_bir_lowering=False, debug=False,
                   enable_asserts=False, num_devices=ncores)

    # ---------------- DRAM I/O ----------------
    states = nc.dram_tensor("states", [A, BC, SD], f32, kind="ExternalInput")
    actions = nc.dram_tensor("actions", [A, BC, AD], f32, kind="ExternalInput")
    wenc = nc.dram_tensor("wenc", [C, 2, F, 64], f32, kind="ExternalInput")
    benc = nc.dram_tensor("benc", [C, 64], f32, kind="ExternalInput")
    wkvs = nc.dram_tensor("wkvs", [65, 128], f32, kind="ExternalInput")
    ident = nc.dram_tensor("ident", [128, 128], bf16, kind="ExternalInput")
    onescol = nc.dram_tensor("onescol", [128, 1], bf16, kind="ExternalInput")

    # kernel-local output layouts (host reassembles):
    #   attn_o [b, i, n, d]; logi_o/prob_o [b, i, j, n] with the full 5x5
    #   (i, j) grid including the (masked) self column.
    attn_o = nc.dram_tensor("attn_o", [BC, C, H, D], f32, kind="ExternalOutput")
    logi_o = nc.dram_tensor("logi_o", [BC, C, C, H], f32,
                            kind="ExternalOutput")
    prob_o = nc.dram_tensor("prob_o", [BC, C, C, H], f32,
                            kind="ExternalOutput")

    with tile.TileContext(nc) as tc, contextlib.ExitStack() as ctx:
        persist = ctx.enter_context(tc.tile_pool(name="persist", bufs=1))
        dram = ctx.enter_context(tc.tile_pool(name="dram", bufs=1, space="DRAM"))
        natp = ctx.enter_context(tc.tile_pool(name="nat", bufs=2))
        nat2p = ctx.enter_context(tc.tile_pool(name="nat2", bufs=1))
        sqp = ctx.enter_context(tc.tile_pool(name="sq", bufs=1))
        tpsum = ctx.enter_context(tc.tile_pool(name="tpsum", bufs=2, space="PSUM"))
        statp = ctx.enter_context(tc.tile_pool(name="statp", bufs=1, space="PSUM"))
        encps = ctx.enter_context(tc.tile_pool(name="encps", bufs=2, space="PSUM"))
        kvsps = ctx.enter_context(tc.tile_pool(name="kvsps", bufs=2, space="PSUM"))
        sasp = ctx.enter_context(tc.tile_pool(name="sas", bufs=2))
        kvbp = ctx.enter_context(tc.tile_pool(name="kvb", bufs=2))
        attp = ctx.enter_context(tc.tile_pool(name="att", bufs=2))
        smallp = ctx.enter_context(tc.tile_pool(name="small", bufs=2))

        # ---------------- persistent SBUF ----------------
        xT = persist.tile([F, A * BC], bf16)        # resident x^T per agent
        ident_sb = persist.tile([128, 128], bf16)
        ones_sb = persist.tile([128, 1], bf16)
        wenc_sb = persist.tile([F, C * 2 * 64], f32)
        wenc_bf = persist.tile([F, C * 2 * 64], bf16)
        benc_sb = persist.tile([64, C], f32)
        wkvs_sb = persist.tile([65, 128], f32)
        wkvs_bf = persist.tile([65, 128], bf16)
        stats_loc = persist.tile([F, 2 * A], f32)
        stats_sb = persist.tile([F, 2 * A], f32)
        m_sb = persist.tile([F, A], f32)
        mbf_sb = persist.tile([F, A], bf16)
        r_sb = persist.tile([F, A], f32)
        tmp_sb = persist.tile([F, A], f32)
        badj = persist.tile([64, C], f32)

        nc.sync.dma_start(ident_sb[:], ident[:, :])
        nc.sync.dma_start(ones_sb[:], onescol[:, :])
        nc.sync.dma_start(
            wenc_sb[:].rearrange("f (c h m) -> f c h m", c=C, h=2),
            wenc.ap().rearrange("c h f m -> f c h m"))
        nc.sync.dma_start(benc_sb[:], benc.ap().rearrange("c m -> m c"))
        nc.sync.dma_start(wkvs_sb[:], wkvs[:, :])
        nc.scalar.activation(wkvs_bf[:], wkvs_sb[:], AF.Copy)

        # stats accumulator [F, 2A]: col 2a = sum_b x, col 2a+1 = sum_b x^2
        stats_ps = statp.tile([F, 2 * A], f32)

        # ---------------- phase 1: load + transpose + stats ----------------
        started = False
        for ch in range(NCH):
            nat = natp.tile([128, NSUB * A * F], bf16)
            nat4 = nat[:].rearrange("p (s a f) -> p s a f", s=NSUB, a=A)
            for sub in range(NSUB):
                b0 = ch * BCH + sub * 128
                nc.gpsimd.dma_start(
                    nat4[:, sub, :, 0:SD],
                    states[:, b0:b0 + 128, :].rearrange("a p f -> p a f"),
                )
                nc.gpsimd.dma_start(
                    nat4[:, sub, :, SD:F],
                    actions[:, b0:b0 + 128, :].rearrange("a p f -> p a f"),
                )
            # bridge copies: every PE consumer of the chunk data then waits on
            # a single DVE semaphore (HW LDW instructions have one wait slot)
            nat2 = nat2p.tile([128, NSUB * A * F], bf16)
            n24 = nat2[:].rearrange("p (s a f) -> p s a f", s=NSUB, a=A)
            for sub in range(NSUB):
                nc.vector.tensor_copy(n24[:, sub, :, 0:SD],
                                      nat4[:, sub, :, 0:SD])
                nc.vector.tensor_copy(n24[:, sub, :, SD:F],
                                      nat4[:, sub, :, SD:F])
            sq = sqp.tile([128, NSUB * A * F], bf16)
            nc.vector.tensor_tensor(sq[:], nat2[:], nat2[:], AluT.mult)
            for a in range(A):
                tp = tpsum.tile([F, NSUB * 128], bf16)
                for sub in range(NSUB):
                    off = (sub * A + a) * F
                    blk = nat2[:, off:off + F]
                    nc.tensor.transpose(tp[:, sub * 128:(sub + 1) * 128],
                                        blk, ident_sb[:])
                    nc.tensor.matmul(
                        stats_ps[:, 2 * a:2 * a + 1], blk, ones_sb[:],
                        start=not started, stop=False)
                    started = True
                    last = (ch == NCH - 1 and sub == NSUB - 1 and a == A - 1)
                    nc.tensor.matmul(
                        stats_ps[:, 2 * a + 1:2 * a + 2], sq[:, off:off + F],
                        ones_sb[:], start=False, stop=last)
                nc.scalar.activation(
                    xT[:, a * BC + ch * BCH: a * BC + (ch + 1) * BCH],
                    tp[:], AF.Copy)

        # ---------------- barrier: AllReduce of stats ----------------
        nc.scalar.activation(stats_loc[:], stats_ps[:], AF.Copy)
        cc_in = dram.tile([F, 2 * A], f32)
        cc_out = dram.tile([F, 2 * A], f32)
        nc.sync.dma_start(cc_in[:], stats_loc[:])
        if single:
            nc.sync.dma_start(cc_out[:], cc_in[:])
        else:
            nc.gpsimd.collective_compute(
                "AllReduce", AluT.add,
                replica_groups=[list(range(NCORES))],
                ins=[cc_in[:].opt()], outs=[cc_out[:].opt()],
            )
        nc.sync.dma_start(stats_sb[:], cc_out[:])

        # m = S1/B ; var = S2/B - m^2 ; r = 1/sqrt(var+eps)
        st3 = stats_sb[:].rearrange("f (a two) -> f a two", two=2)
        nc.vector.tensor_scalar(m_sb[:], st3[:, :, 0], 1.0 / B, None, AluT.mult)
        nc.vector.tensor_tensor(tmp_sb[:], m_sb[:], m_sb[:], AluT.mult)
        nc.vector.tensor_scalar(r_sb[:], st3[:, :, 1], 1.0 / B, None, AluT.mult)
        nc.vector.tensor_tensor(r_sb[:], r_sb[:], tmp_sb[:], AluT.subtract)
        nc.vector.tensor_scalar(r_sb[:], r_sb[:], BN_EPS, None, AluT.add)
        nc.scalar.activation(tmp_sb[:], r_sb[:], AF.Sqrt)
        nc.vector.reciprocal(r_sb[:], tmp_sb[:])
        nc.vector.tensor_copy(mbf_sb[:], m_sb[:])

        # W' = W * r (per-feature row scale), cast to bf16
        for c in range(C):
            for h2 in range(2):
                a = 2 * c + h2
                sl = slice((c * 2 + h2) * 64, (c * 2 + h2) * 64 + 64)
                nc.scalar.activation(wenc_bf[:, sl], wenc_sb[:, sl], AF.Copy,
                                     scale=r_sb[:, a:a + 1])
        # badj[c] = benc[c] - m^T @ W'
        for c in range(C):
            bps = encps.tile([64, BCH], f32, tag="eps")
            for h2 in range(2):
                a = 2 * c + h2
                sl = slice((c * 2 + h2) * 64, (c * 2 + h2) * 64 + 64)
                nc.tensor.matmul(bps[:, 0:1], wenc_bf[:, sl],
                                 mbf_sb[:, a:a + 1],
                                 start=(h2 == 0), stop=(h2 == 1))
            nc.scalar.activation(badj[:, c:c + 1], bps[:, 0:1], AF.Identity,
                                 bias=benc_sb[:, c:c + 1], scale=-1.0)

        # ---------------- phase 2: encoder + attention ----------------
        for ch in range(NCH):
            # KVSb: [128b, (sub, c, t)]  t: k 0-31 | v 32-63 | v001 64-95 |
            # sel 96-127
            kvsb = kvbp.tile([128, NSUB * C * 128], bf16)
            kv4 = kvsb[:].rearrange("p (s c t) -> p s c t", s=NSUB, c=C)
            for c in range(C):
                eps_t = encps.tile([64, BCH], f32, tag="eps")
                for h2 in range(2):
                    a = 2 * c + h2
                    sl = slice((c * 2 + h2) * 64, (c * 2 + h2) * 64 + 64)
                    nc.tensor.matmul(
                        eps_t[:], wenc_bf[:, sl],
                        xT[:, a * BC + ch * BCH: a * BC + (ch + 1) * BCH],
                        start=(h2 == 0), stop=(h2 == 1))
                sas = sasp.tile([65, BCH], bf16)
                nc.scalar.activation(sas[0:64, :], eps_t[:], AF.Identity,
                                     bias=badj[:, c:c + 1])
                z = sasp.tile([64, BCH], bf16, tag="zt")
                nc.vector.tensor_scalar(z[:], sas[0:64, :], 0.01, None,
                                        AluT.mult)
                nc.vector.tensor_tensor(sas[0:64, :], sas[0:64, :], z[:],
                                        AluT.max)
                nc.vector.memset(sas[64:65, :], 1.0)
                # kvs in batch-major: out[128b, 128t] = sas_sub^T @ wkvs
                kps = kvsps.tile([128, NSUB * 128], f32)
                for sub in range(NSUB):
                    nc.tensor.matmul(
                        kps[:, sub * 128:(sub + 1) * 128],
                        sas[:, sub * 128:(sub + 1) * 128], wkvs_bf[:],
                        start=(sub == 0), stop=(sub == NSUB - 1))
                nc.scalar.activation(
                    kv4[:, :, c, :],
                    kps[:].rearrange("p (s t) -> p s t", s=NSUB), AF.Copy)

            # vals leaky-relu: v = max(v+b, 0.01*(v+b)) via the v001 columns
            nc.vector.tensor_tensor(kv4[:, :, :, 32:64], kv4[:, :, :, 32:64],
                                    kv4[:, :, :, 64:96], AluT.max)

            # logits products: lp[128, (sub, i, j, nd)] = sel_i * k_j
            lp = attp.tile([128, NSUB * C * C * ND], bf16, tag="lp")
            lp5 = lp[:].rearrange("p (s i j t) -> p s i j t", s=NSUB, i=C, j=C)
            for sub in range(NSUB):
                sel_v = (kv4[:, sub, :, 96:128]
                         .rearrange("p i (one t) -> p i one t", one=1)
                         .broadcast_to([128, C, C, ND]))
                key_v = (kv4[:, sub, :, 0:32]
                         .rearrange("p (one j) t -> p one j t", one=1)
                         .broadcast_to([128, C, C, ND]))
                nc.vector.tensor_tensor(lp5[:, sub], sel_v, key_v, AluT.mult)
            # d-contraction 8 -> 1 by halving adds; layout (q=(s,i,j), n, d)
            lpn = lp[:].rearrange("p (q n d) -> p q n d", n=H, d=D)
            l1 = attp.tile([128, NSUB * C * C * H * 4], bf16, tag="l1")
            l1v = l1[:].rearrange("p (q n d) -> p q n d", n=H, d=4)
            nc.vector.tensor_tensor(l1v, lpn[:, :, :, 0:4], lpn[:, :, :, 4:8],
                                    AluT.add)
            l2 = attp.tile([128, NSUB * C * C * H * 2], bf16, tag="l2")
            l2v = l2[:].rearrange("p (q n d) -> p q n d", n=H, d=2)
            nc.vector.tensor_tensor(l2v, l1v[:, :, :, 0:2], l1v[:, :, :, 2:4],
                                    AluT.add)
            lgt = attp.tile([128, NSUB * C * C * H], f32, tag="lgt")
            lgtv = lgt[:].rearrange("p (q n d) -> p q n d", n=H, d=1)
            nc.vector.tensor_tensor(lgtv, l2v[:, :, :, 0:1], l2v[:, :, :, 1:2],
                                    AluT.add)

            # self mask: slots k = i*(C+1) in the (i,j) grid
            mask_v = (lgt[:].rearrange("p (s k n) -> p s k n", s=NSUB, k=C * C)
                      [:, :, 0:C * C:C + 1, :])
            nc.vector.memset(mask_v, -1e30)

            # exp((logits - 0)/sqrt(D)) -> bf16 (no max-subtraction needed:
            # |logits| is O(1) for this model)
            expt = attp.tile([128, NSUB * C * C * H], bf16, tag="expt")
            nc.scalar.activation(expt[:], lgt[:], AF.Exp, scale=INV_SQRT_D)

            ex5 = expt[:].rearrange("p (s i j n) -> p s i j n", s=NSUB, i=C,
                                    j=C)
            exq = expt[:].rearrange("p (q j n) -> p q j n", j=C, n=H)
            # denominator: sum over j (5 = 2+2+1)
            r1 = smallp.tile([128, NSUB * C * 2 * H], bf16, tag="r1")
            r1v = r1[:].rearrange("p (q j n) -> p q j n", j=2, n=H)
            nc.vector.tensor_tensor(r1v, exq[:, :, 0:2, :], exq[:, :, 2:4, :],
                                    AluT.add)
            rs = smallp.tile([128, NSUB * C * H], f32, tag="rs")
            rsv = rs[:].rearrange("p (q one n) -> p q one n", one=1, n=H)
            nc.vector.tensor_tensor(rsv, r1v[:, :, 0:1, :], r1v[:, :, 1:2, :],
                                    AluT.add)
            nc.vector.tensor_tensor(rsv, rsv, exq[:, :, 4:5, :], AluT.add)
            rc = smallp.tile([128, NSUB * C * H], f32, tag="rc")
            nc.vector.reciprocal(rc[:], rs[:])

            # probs (fp32 output): expt * rc broadcast over j
            probs = attp.tile([128, NSUB * C * C * H], f32, tag="probs")
            rcb = (rc[:].rearrange("p (q n) -> p q n", n=H)
                   .rearrange("p q (one n) -> p q one n", one=1)
                   .broadcast_to([128, NSUB * C, C, H]))
            nc.vector.tensor_tensor(
                probs[:].rearrange("p (q j n) -> p q j n", j=C, n=H),
                exq, rcb, AluT.mult)

            # attention products: at[(sub,i,j,(n,d))] = expt(i,j,n) * v(j,n,d)
            at = attp.tile([128, NSUB * C * C * ND], bf16, tag="at")
            at5 = at[:].rearrange("p (s i j n d) -> p s i j n d", s=NSUB, i=C,
                                  j=C, n=H)
            for sub in range(NSUB):
                e_v = (ex5[:, sub]
                       .rearrange("p i j (n one) -> p i j n one", one=1)
                       .broadcast_to([128, C, C, H, D]))
                v_v = (kv4[:, sub, :, 32:64]
                       .rearrange("p (one j) t -> p one j t", one=1)
                       .broadcast_to([128, C, C, ND])
                       .rearrange("p i j (n d) -> p i j n d", n=H))
                nc.vector.tensor_tensor(at5[:, sub], e_v, v_v, AluT.mult)
            # j-contraction (5 = 2+2+1)
            atq = at[:].rearrange("p (q j t) -> p q j t", j=C, t=ND)
            a1 = attp.tile([128, NSUB * C * 2 * ND], bf16, tag="a1")
            a1v = a1[:].rearrange("p (q j t) -> p q j t", j=2, t=ND)
            nc.vector.tensor_tensor(a1v, atq[:, :, 0:2, :], atq[:, :, 2:4, :],
                                    AluT.add)
            au = attp.tile([128, NSUB * C * ND], bf16, tag="au")
            auv = au[:].rearrange("p (q one t) -> p q one t", one=1, t=ND)
            nc.vector.tensor_tensor(auv, a1v[:, :, 0:1, :], a1v[:, :, 1:2, :],
                                    AluT.add)
            nc.vector.tensor_tensor(auv, auv, atq[:, :, 4:5, :], AluT.add)
            # final scale by rc broadcast over d -> fp32
            attn = attp.tile([128, NSUB * C * ND], f32, tag="attn")
            rcd = (rc[:].rearrange("p (q n) -> p q n", n=H)
                   .rearrange("p q (n one) -> p q n one", one=1)
                   .broadcast_to([128, NSUB * C, H, D]))
            nc.vector.tensor_tensor(
                attn[:].rearrange("p (q n d) -> p q n d", n=H, d=D),
                au[:].rearrange("p (q n d) -> p q n d", n=H, d=D), rcd,
                AluT.mult)

            # ---------------- output DMAs (HWDGE) ----------------
            nc.sync.dma_start(
                attn_o[ch * BCH:(ch + 1) * BCH]
                .rearrange("(s p) i n d -> p s (i n d)", p=128),
                attn[:].rearrange("p (s x) -> p s x", s=NSUB),
            )
            nc.sync.dma_start(
                logi_o[ch * BCH:(ch + 1) * BCH]
                .rearrange("(s p) i j n -> p s (i j n)", p=128),
                lgt[:].rearrange("p (s x) -> p s x", s=NSUB),
            )
            nc.sync.dma_start(
                prob_o[ch * BCH:(ch + 1) * BCH]
                .rearrange("(s p) i j n -> p s (i j n)", p=128),
                probs[:].rearrange("p (s x) -> p s x", s=NSUB),
            )
    nc.compile()
    return nc


def _host_inputs(inputs):
    """Slice batch per core + prepack weights into device layouts."""
    states = np.asarray(inputs["states"], np.float32)
    actions = np.asarray(inputs["actions"], np.float32)
    enc_W = np.asarray(inputs["enc_W"], np.float32)     # [C, HID, 240]
    enc_b = np.asarray(inputs["enc_b"], np.float32)     # [C, HID]
    senc_W = np.asarray(inputs["senc_W"], np.float32)   # [C, HID, 200]
    senc_b = np.asarray(inputs["senc_b"], np.float32)   # [C, HID]
    key_W = np.asarray(inputs["key_W"], np.float32)     # [H, D, HID]
    sel_W = np.asarray(inputs["sel_W"], np.float32)
    val_W = np.asarray(inputs["val_W"], np.float32)
    val_b = np.asarray(inputs["val_b"], np.float32)     # [H, D]

    wenc = np.zeros((C, 2, F, 64), np.float32)
    for c in range(C):
        for h2 in range(2):
            wsa = np.concatenate(
                [enc_W[c, :, h2 * SD:(h2 + 1) * SD],
                 enc_W[c, :, 2 * SD + h2 * AD: 2 * SD + (h2 + 1) * AD]],
                axis=1)                                   # [32, 120]
            ws = senc_W[c, :, h2 * SD:(h2 + 1) * SD]      # [32, 100]
            wenc[c, h2, :, 0:32] = wsa.T
            wenc[c, h2, 0:SD, 32:64] = ws.T
    benc = np.ascontiguousarray(
        np.concatenate([enc_b, senc_b], axis=1))          # [C, 64]

    wkvs = np.zeros((65, 128), np.float32)
    kw = key_W.reshape(ND, HID).T
    vw = val_W.reshape(ND, HID).T
    sw = sel_W.reshape(ND, HID).T
    vb = val_b.reshape(ND)
    wkvs[0:32, 0:32] = kw
    wkvs[0:32, 32:64] = vw
    wkvs[0:32, 64:96] = 0.01 * vw
    wkvs[32:64, 96:128] = sw
    wkvs[64, 32:64] = vb
    wkvs[64, 64:96] = 0.01 * vb

    ident = np.eye(128, dtype=ml_dtypes.bfloat16)
    ones = np.ones((128, 1), ml_dtypes.bfloat16)

    in_maps = []
    for core in range(NCORES):
        sl = slice(core * BC, (core + 1) * BC)
        in_maps.append({
            "states": np.ascontiguousarray(states[:, sl, :]),
            "actions": np.ascontiguousarray(actions[:, sl, :]),
            "wenc": wenc, "benc": benc, "wkvs": wkvs,
            "ident": ident, "onescol": ones,
        })
    return in_maps


def kernel(**inputs):
    global _BUILT, LAST_RESULTS
    from concourse import bass_utils

    if _BUILT is None:
        _BUILT = _build_bass()
    nc = _BUILT

    in_maps = _host_inputs(inputs)
    res = bass_utils.run_bass_kernel_spmd(
        nc, in_maps, core_ids=list(range(NCORES)), trace=TRACE)
    LAST_RESULTS = res

    others = np.array([[j for j in range(C) if j != i] for i in range(C)])
    attn_b = np.concatenate([res.results[c]["attn_o"] for c in range(NCORES)],
                            axis=0)                      # [B, C, H, D]
    logi_b = np.concatenate([res.results[c]["logi_o"] for c in range(NCORES)],
                            axis=0)                      # [B, C, C, H]
    prob_b = np.concatenate([res.results[c]["prob_o"] for c in range(NCORES)],
                            axis=0)
    attn = np.ascontiguousarray(attn_b.transpose(1, 2, 0, 3))
    idx = others[None, :, :, None]                       # [1, C, C-1, 1]
    logi = np.take_along_axis(logi_b, idx, axis=2).transpose(1, 3, 0, 2)
    prob = np.take_along_axis(prob_b, idx, axis=2).transpose(1, 3, 0, 2)
    return (attn, np.ascontiguousarray(logi), np.ascontiguousarray(prob))


# revision 14
# speedup vs baseline: 1.0373x; 1.0373x over previous
"""Trainium2 Bass kernel for nn_ClusterCritic (gnn_message_passing).

Data-parallel over 8 NeuronCores: batch axis B=32768 sharded 8x4096.
Per core, two phases:
  phase 1: stream inputs (fp32->bf16 cast DMA), PE-transpose to feature-major
           x^T tiles resident in SBUF, accumulate per-feature sum / sum-of-
           squares in PSUM via PE matmuls (batch is the contraction axis).
  barrier: AllReduce of the (sum, sumsq) stats across the 8 cores, then fold
           BatchNorm into the encoder weights (W' = W*rsqrt(var+eps),
           b' = b - m @ W'^T) on device.
  phase 2: per 512-batch chunk: fused sa/s encoder matmuls -> bias+leaky-relu
           -> fused keys/vals/sel matmul emitted directly in batch-major
           (activation as the stationary operand) -> leave-one-out attention
           (products + d/j tree-contractions on DVE, exp on ACT) -> outputs.
"""

import numpy as np
import ml_dtypes

# ---- static problem configuration (hardcoded per contract) ----
A, C, PER = 10, 5, 2
B = 32768
SD, AD = 100, 20
F = SD + AD            # 120 features per agent
HID, H, D = 32, 4, 8
ND = H * D             # 32
NCORES = 8
BC = B // NCORES       # 4096 batch per core
NCH = 8                # chunks per core
BCH = BC // NCH        # 512 batch per chunk
NSUB = 4               # 128-row subtiles per chunk
F2 = 128           # agent feature block padded for FWL
BN_EPS = 1e-5
INV_SQRT_D = float(1.0 / np.sqrt(D))

_BUILT = None  # cached Bass module
TRACE = False
LAST_RESULTS = None


def _build_bass(single=False):
    import contextlib
    import concourse.bass as bass
    import concourse.tile as tile
    from concourse import bacc, mybir

    f32 = mybir.dt.float32
    bf16 = mybir.dt.bfloat16
    AluT = mybir.AluOpType
    AF = mybir.ActivationFunctionType

    ncores = 1 if single else NCORES
    nc = bacc.Bacc("TRN2", target_bir_lowering=False, debug=False,
                   enable_asserts=False, num_devices=ncores)

    # ---------------- DRAM I/O ----------------
    states = nc.dram_tensor("states", [A, BC, SD], f32, kind="ExternalInput")
    actions = nc.dram_tensor("actions", [A, BC, AD], f32, kind="ExternalInput")
    wenc = nc.dram_tensor("wenc", [C, 2, F, 64], f32, kind="ExternalInput")
    benc = nc.dram_tensor("benc", [C, 64], f32, kind="ExternalInput")
    wkvs = nc.dram_tensor("wkvs", [65, 128], f32, kind="ExternalInput")
    ident = nc.dram_tensor("ident", [128, 128], bf16, kind="ExternalInput")
    onescol = nc.dram_tensor("onescol", [128, 1], bf16, kind="ExternalInput")

    # kernel-local output layouts (host reassembles):
    #   attn_o [b, i, n, d]; logi_o/prob_o [b, i, j, n] with the full 5x5
    #   (i, j) grid including the (masked) self column.
    attn_o = nc.dram_tensor("attn_o", [BC, C, H, D], f32, kind="ExternalOutput")
    logi_o = nc.dram_tensor("logi_o", [BC, C, C, H], f32,
                            kind="ExternalOutput")
    prob_o = nc.dram_tensor("prob_o", [BC, C, C, H], f32,
                            kind="ExternalOutput")

    with tile.TileContext(nc) as tc, contextlib.ExitStack() as ctx:
        persist = ctx.enter_context(tc.tile_pool(name="persist", bufs=1))
        dram = ctx.enter_context(tc.tile_pool(name="dram", bufs=1, space="DRAM"))
        natp = ctx.enter_context(tc.tile_pool(name="nat", bufs=2))
        sqp = ctx.enter_context(tc.tile_pool(name="sq", bufs=2))
        tpsum = ctx.enter_context(tc.tile_pool(name="tpsum", bufs=2, space="PSUM"))
        statp = ctx.enter_context(tc.tile_pool(name="statp", bufs=1, space="PSUM"))
        encps = ctx.enter_context(tc.tile_pool(name="encps", bufs=2, space="PSUM"))
        kvsps = ctx.enter_context(tc.tile_pool(name="kvsps", bufs=2, space="PSUM"))
        sasp = ctx.enter_context(tc.tile_pool(name="sas", bufs=2))
        kvbp = ctx.enter_context(tc.tile_pool(name="kvb", bufs=2))
        attp = ctx.enter_context(tc.tile_pool(name="att", bufs=2))
        smallp = ctx.enter_context(tc.tile_pool(name="small", bufs=2))

        # ---------------- persistent SBUF ----------------
        xT = persist.tile([F, A * BC], bf16)        # resident x^T per agent
        ident_sb = persist.tile([128, 128], bf16)
        ones_sb = persist.tile([128, 1], bf16)
        wenc_sb = persist.tile([F, C * 2 * 64], f32)
        wenc_bf = persist.tile([F, C * 2 * 64], bf16)
        benc_sb = persist.tile([64, C], f32)
        wkvs_sb = persist.tile([65, 128], f32)
        wkvs_bf = persist.tile([65, 128], bf16)
        stats_loc = persist.tile([F, 2 * A], f32)
        stats_sb = persist.tile([F, 2 * A], f32)
        m_sb = persist.tile([F, A], f32)
        mbf_sb = persist.tile([F, A], bf16)
        r_sb = persist.tile([F, A], f32)
        tmp_sb = persist.tile([F, A], f32)
        badj = persist.tile([64, C], f32)
        sas2 = [persist.tile([65, BCH], bf16, name=f"sas{i}", tag=f"sas{i}")
                for i in range(2)]

        nc.sync.dma_start(ident_sb[:], ident[:, :])
        nc.sync.dma_start(ones_sb[:], onescol[:, :])
        nc.sync.dma_start(
            wenc_sb[:].rearrange("f (c h m) -> f c h m", c=C, h=2),
            wenc.ap().rearrange("c h f m -> f c h m"))
        nc.sync.dma_start(benc_sb[:], benc.ap().rearrange("c m -> m c"))
        nc.sync.dma_start(wkvs_sb[:], wkvs[:, :])
        nc.scalar.activation(wkvs_bf[:], wkvs_sb[:], AF.Copy)
        nc.vector.memset(sas2[0][64:65, :], 1.0)
        nc.vector.memset(sas2[1][64:65, :], 1.0)

        # stats accumulator [F, 2A]: col 2a = sum_b x, col 2a+1 = sum_b x^2
        stats_ps = statp.tile([128, 2 * A], f32)

        # ---------------- phase 1: load + transpose + stats ----------------
        started = False
        for ch in range(NCH):
            nat = natp.tile([128, NSUB * A * F2], bf16)
            nat4 = nat[:].rearrange("p (s a f) -> p s a f", s=NSUB, a=A)
            nc.gpsimd.memset(nat4[:, :, :, F:F2], 0.0)
            for sub in range(NSUB):
                b0 = ch * BCH + sub * 128
                nc.gpsimd.dma_start(
                    nat4[:, sub, :, 0:SD],
                    states[:, b0:b0 + 128, :].rearrange("a p f -> p a f"),
                )
                nc.gpsimd.dma_start(
                    nat4[:, sub, :, SD:F],
                    actions[:, b0:b0 + 128, :].rearrange("a p f -> p a f"),
                )
            sq = sqp.tile([128, NSUB * A * F2], bf16)
            nc.vector.tensor_tensor(sq[:], nat[:], nat[:], AluT.mult)
            for a in range(A):
                tp = tpsum.tile([128, NSUB * 128], bf16)
                for sub in range(NSUB):
                    off = (sub * A + a) * F2
                    blk = nat[:, off:off + F2]
                    nc.tensor.transpose(tp[:, sub * 128:(sub + 1) * 128],
                                        blk, ident_sb[:])
                    nc.tensor.matmul(
                        stats_ps[:, 2 * a:2 * a + 1], blk, ones_sb[:],
                        start=not started, stop=False)
                    started = True
                    last = (ch == NCH - 1 and sub == NSUB - 1 and a == A - 1)
                    nc.tensor.matmul(
                        stats_ps[:, 2 * a + 1:2 * a + 2], sq[:, off:off + F2],
                        ones_sb[:], start=False, stop=last)
                nc.scalar.activation(
                    xT[:, a * BC + ch * BCH: a * BC + (ch + 1) * BCH],
                    tp[0:F, :], AF.Copy)

        # ---------------- barrier: AllReduce of stats ----------------
        nc.scalar.activation(stats_loc[:], stats_ps[0:F, :], AF.Copy)
        cc_in = dram.tile([F, 2 * A], f32)
        cc_out = dram.tile([F, 2 * A], f32)
        nc.sync.dma_start(cc_in[:], stats_loc[:])
        if single:
            nc.sync.dma_start(cc_out[:], cc_in[:])
        else:
            nc.gpsimd.collective_compute(
                "AllReduce", AluT.add,
                replica_groups=[list(range(NCORES))],
                ins=[cc_in[:].opt()], outs=[cc_out[:].opt()],
            )
        nc.sync.dma_start(stats_sb[:], cc_out[:])

        # m = S1/B ; var = S2/B - m^2 ; r = 1/sqrt(var+eps)
        st3 = stats_sb[:].rearrange("f (a two) -> f a two", two=2)
        nc.vector.tensor_scalar(m_sb[:], st3[:, :, 0], 1.0 / B, None, AluT.mult)
        nc.vector.tensor_tensor(tmp_sb[:], m_sb[:], m_sb[:], AluT.mult)
        nc.vector.tensor_scalar(r_sb[:], st3[:, :, 1], 1.0 / B, None, AluT.mult)
        nc.vector.tensor_tensor(r_sb[:], r_sb[:], tmp_sb[:], AluT.subtract)
        nc.vector.tensor_scalar(r_sb[:], r_sb[:], BN_EPS, None, AluT.add)
        nc.scalar.activation(tmp_sb[:], r_sb[:], AF.Sqrt)
        nc.vector.reciprocal(r_sb[:], tmp_sb[:])
        nc.vector.tensor_copy(mbf_sb[:], m_sb[:])

        # W' = W * r (per-feature row scale), cast to bf16
        for c in range(C):
            for h2 in range(2):
                a = 2 * c + h2
                sl = slice((c * 2 + h2) * 64, (c * 2 + h2) * 64 + 64)
                nc.scalar.activation(wenc_bf[:, sl], wenc_sb[:, sl], AF.Copy,
                                     scale=r_sb[:, a:a + 1])
        # badj[c] = benc[c] - m^T @ W'
        for c in range(C):
            bps = encps.tile([64, BCH], f32, tag="eps")
            for h2 in range(2):
                a = 2 * c + h2
                sl = slice((c * 2 + h2) * 64, (c * 2 + h2) * 64 + 64)
                nc.tensor.matmul(bps[:, 0:1], wenc_bf[:, sl],
                                 mbf_sb[:, a:a + 1],
                                 start=(h2 == 0), stop=(h2 == 1))
            nc.scalar.activation(badj[:, c:c + 1], bps[:, 0:1], AF.Identity,
                                 bias=benc_sb[:, c:c + 1], scale=-1.0)

        # ---------------- phase 2: encoder + attention ----------------
        for ch in range(NCH):
            # KVSb: [128b, (sub, c, t)]  t: k 0-31 | v 32-63 | v001 64-95 |
            # sel 96-127
            kvsb = kvbp.tile([128, NSUB * C * 128], bf16)
            kv4 = kvsb[:].rearrange("p (s c t) -> p s c t", s=NSUB, c=C)
            for c in range(C):
                eps_t = encps.tile([64, BCH], f32, tag="eps")
                for h2 in range(2):
                    a = 2 * c + h2
                    sl = slice((c * 2 + h2) * 64, (c * 2 + h2) * 64 + 64)
                    nc.tensor.matmul(
                        eps_t[:], wenc_bf[:, sl],
                        xT[:, a * BC + ch * BCH: a * BC + (ch + 1) * BCH],
                        start=(h2 == 0), stop=(h2 == 1))
                sas = sas2[c % 2]
                nc.scalar.activation(sas[0:64, :], eps_t[:], AF.Identity,
                                     bias=badj[:, c:c + 1])
                z = sasp.tile([64, BCH], bf16, tag="zt")
                nc.vector.tensor_scalar(z[:], sas[0:64, :], 0.01, None,
                                        AluT.mult)
                nc.vector.tensor_tensor(sas[0:64, :], sas[0:64, :], z[:],
                                        AluT.max)
                # kvs in batch-major: out[128b, 128t] = sas_sub^T @ wkvs
                kps = kvsps.tile([128, NSUB * 128], f32)
                for sub in range(NSUB):
                    nc.tensor.matmul(
                        kps[:, sub * 128:(sub + 1) * 128],
                        sas[:, sub * 128:(sub + 1) * 128], wkvs_bf[:],
                        start=(sub == 0), stop=(sub == NSUB - 1))
                nc.scalar.activation(
                    kv4[:, :, c, :],
                    kps[:].rearrange("p (s t) -> p s t", s=NSUB), AF.Copy)

            # vals leaky-relu: v = max(v+b, 0.01*(v+b)) via the v001 columns
            nc.vector.tensor_tensor(kv4[:, :, :, 32:64], kv4[:, :, :, 32:64],
                                    kv4[:, :, :, 64:96], AluT.max)

            # logits products: lp[128, (sub, i, j, nd)] = sel_i * k_j
            lp = attp.tile([128, NSUB * C * C * ND], bf16, tag="lp")
            lp5 = lp[:].rearrange("p (s i j t) -> p s i j t", s=NSUB, i=C, j=C)
            for sub in range(NSUB):
                sel_v = (kv4[:, sub, :, 96:128]
                         .rearrange("p i (one t) -> p i one t", one=1)
                         .broadcast_to([128, C, C, ND]))
                key_v = (kv4[:, sub, :, 0:32]
                         .rearrange("p (one j) t -> p one j t", one=1)
                         .broadcast_to([128, C, C, ND]))
                nc.vector.tensor_tensor(lp5[:, sub], sel_v, key_v, AluT.mult)
            # d-contraction 8 -> 1 by halving adds; layout (q=(s,i,j), n, d)
            lpn = lp[:].rearrange("p (q n d) -> p q n d", n=H, d=D)
            l1 = attp.tile([128, NSUB * C * C * H * 4], bf16, tag="l1")
            l1v = l1[:].rearrange("p (q n d) -> p q n d", n=H, d=4)
            nc.vector.tensor_tensor(l1v, lpn[:, :, :, 0:4], lpn[:, :, :, 4:8],
                                    AluT.add)
            l2 = attp.tile([128, NSUB * C * C * H * 2], bf16, tag="l2")
            l2v = l2[:].rearrange("p (q n d) -> p q n d", n=H, d=2)
            nc.vector.tensor_tensor(l2v, l1v[:, :, :, 0:2], l1v[:, :, :, 2:4],
                                    AluT.add)
            lgt = attp.tile([128, NSUB * C * C * H], f32, tag="lgt")
            lgtv = lgt[:].rearrange("p (q n d) -> p q n d", n=H, d=1)
            nc.vector.tensor_tensor(lgtv, l2v[:, :, :, 0:1], l2v[:, :, :, 1:2],
                                    AluT.add)

            # self mask: slots k = i*(C+1) in the (i,j) grid
            mask_v = (lgt[:].rearrange("p (s k n) -> p s k n", s=NSUB, k=C * C)
                      [:, :, 0:C * C:C + 1, :])
            nc.vector.memset(mask_v, -1e30)

            # exp((logits - 0)/sqrt(D)) -> bf16 (no max-subtraction needed:
            # |logits| is O(1) for this model)
            expt = attp.tile([128, NSUB * C * C * H], bf16, tag="expt")
            nc.scalar.activation(expt[:], lgt[:], AF.Exp, scale=INV_SQRT_D)

            ex5 = expt[:].rearrange("p (s i j n) -> p s i j n", s=NSUB, i=C,
                                    j=C)
            exq = expt[:].rearrange("p (q j n) -> p q j n", j=C, n=H)
            # denominator: sum over j (5 = 2+2+1)
            r1 = smallp.tile([128, NSUB * C * 2 * H], bf16, tag="r1")
            r1v = r1[:].rearrange("p (q j n) -> p q j n", j=2, n=H)
            nc.vector.tensor_tensor(r1v, exq[:, :, 0:2, :], exq[:, :, 2:4, :],
                                    AluT.add)
            rs = smallp.tile([128, NSUB * C * H], f32, tag="rs")
            rsv = rs[:].rearrange("p (q one n) -> p q one n", one=1, n=H)
            nc.vector.tensor_tensor(rsv, r1v[:, :, 0:1, :], r1v[:, :, 1:2, :],
                                    AluT.add)
            nc.vector.tensor_tensor(rsv, rsv, exq[:, :, 4:5, :], AluT.add)
            rc = smallp.tile([128, NSUB * C * H], f32, tag="rc")
            nc.vector.reciprocal(rc[:], rs[:])

            # probs (fp32 output): expt * rc broadcast over j
            probs = attp.tile([128, NSUB * C * C * H], f32, tag="probs")
            rcb = (rc[:].rearrange("p (q n) -> p q n", n=H)
                   .rearrange("p q (one n) -> p q one n", one=1)
                   .broadcast_to([128, NSUB * C, C, H]))
            nc.vector.tensor_tensor(
                probs[:].rearrange("p (q j n) -> p q j n", j=C, n=H),
                exq, rcb, AluT.mult)

            # attention products: at[(sub,i,j,(n,d))] = expt(i,j,n) * v(j,n,d)
            at = attp.tile([128, NSUB * C * C * ND], bf16, tag="at")
            at5 = at[:].rearrange("p (s i j n d) -> p s i j n d", s=NSUB, i=C,
                                  j=C, n=H)
            for sub in range(NSUB):
                e_v = (ex5[:, sub]
                       .rearrange("p i j (n one) -> p i j n one", one=1)
                       .broadcast_to([128, C, C, H, D]))
                v_v = (kv4[:, sub, :, 32:64]
                       .rearrange("p (one j) t -> p one j t", one=1)
                       .broadcast_to([128, C, C, ND])
                       .rearrange("p i j (n d) -> p i j n d", n=H))
                nc.vector.tensor_tensor(at5[:, sub], e_v, v_v, AluT.mult)
            # j-contraction (5 = 2+2+1)
            atq = at[:].rearrange("p (q j t) -> p q j t", j=C, t=ND)
            a1 = attp.tile([128, NSUB * C * 2 * ND], bf16, tag="a1")
            a1v = a1[:].rearrange("p (q j t) -> p q j t", j=2, t=ND)
            nc.vector.tensor_tensor(a1v, atq[:, :, 0:2, :], atq[:, :, 2:4, :],
                                    AluT.add)
            au = attp.tile([128, NSUB * C * ND], bf16, tag="au")
            auv = au[:].rearrange("p (q one t) -> p q one t", one=1, t=ND)
            nc.vector.tensor_tensor(auv, a1v[:, :, 0:1, :], a1v[:, :, 1:2, :],
                                    AluT.add)
            nc.vector.tensor_tensor(auv, auv, atq[:, :, 4:5, :], AluT.add)
            # final scale by rc broadcast over d -> fp32
            attn = attp.tile([128, NSUB * C * ND], f32, tag="attn")
            rcd = (rc[:].rearrange("p (q n) -> p q n", n=H)
                   .rearrange("p q (n one) -> p q n one", one=1)
                   .broadcast_to([128, NSUB * C, H, D]))
            nc.vector.tensor_tensor(
                attn[:].rearrange("p (q n d) -> p q n d", n=H, d=D),
                au[:].rearrange("p (q n d) -> p q n d", n=H, d=D), rcd,
                AluT.mult)

            # ---------------- output DMAs (HWDGE) ----------------
            nc.sync.dma_start(
                attn_o[ch * BCH:(ch + 1) * BCH]
                .rearrange("(s p) i n d -> p s (i n d)", p=128),
                attn[:].rearrange("p (s x) -> p s x", s=NSUB),
            )
            nc.sync.dma_start(
                logi_o[ch * BCH:(ch + 1) * BCH]
                .rearrange("(s p) i j n -> p s (i j n)", p=128),
                lgt[:].rearrange("p (s x) -> p s x", s=NSUB),
            )
            nc.sync.dma_start(
                prob_o[ch * BCH:(ch + 1) * BCH]
                .rearrange("(s p) i j n -> p s (i j n)", p=128),
                probs[:].rearrange("p (s x) -> p s x", s=NSUB),
            )
    nc.compile()
    return nc


def _host_inputs(inputs):
    """Slice batch per core + prepack weights into device layouts."""
    states = np.asarray(inputs["states"], np.float32)
    actions = np.asarray(inputs["actions"], np.float32)
    enc_W = np.asarray(inputs["enc_W"], np.float32)     # [C, HID, 240]
    enc_b = np.asarray(inputs["enc_b"], np.float32)     # [C, HID]
    senc_W = np.asarray(inputs["senc_W"], np.float32)   # [C, HID, 200]
    senc_b = np.asarray(inputs["senc_b"], np.float32)   # [C, HID]
    key_W = np.asarray(inputs["key_W"], np.float32)     # [H, D, HID]
    sel_W = np.asarray(inputs["sel_W"], np.float32)
    val_W = np.asarray(inputs["val_W"], np.float32)
    val_b = np.asarray(inputs["val_b"], np.float32)     # [H, D]

    wenc = np.zeros((C, 2, F, 64), np.float32)
    for c in range(C):
        for h2 in range(2):
            wsa = np.concatenate(
                [enc_W[c, :, h2 * SD:(h2 + 1) * SD],
                 enc_W[c, :, 2 * SD + h2 * AD: 2 * SD + (h2 + 1) * AD]],
                axis=1)                                   # [32, 120]
            ws = senc_W[c, :, h2 * SD:(h2 + 1) * SD]      # [32, 100]
            wenc[c, h2, :, 0:32] = wsa.T
            wenc[c, h2, 0:SD, 32:64] = ws.T
    benc = np.ascontiguousarray(
        np.concatenate([enc_b, senc_b], axis=1))          # [C, 64]

    wkvs = np.zeros((65, 128), np.float32)
    kw = key_W.reshape(ND, HID).T
    vw = val_W.reshape(ND, HID).T
    sw = sel_W.reshape(ND, HID).T
    vb = val_b.reshape(ND)
    wkvs[0:32, 0:32] = kw
    wkvs[0:32, 32:64] = vw
    wkvs[0:32, 64:96] = 0.01 * vw
    wkvs[32:64, 96:128] = sw
    wkvs[64, 32:64] = vb
    wkvs[64, 64:96] = 0.01 * vb

    ident = np.eye(128, dtype=ml_dtypes.bfloat16)
    ones = np.ones((128, 1), ml_dtypes.bfloat16)

    in_maps = []
    for core in range(NCORES):
        sl = slice(core * BC, (core + 1) * BC)
        in_maps.append({
            "states": np.ascontiguousarray(states[:, sl, :]),
            "actions": np.ascontiguousarray(actions[:, sl, :]),
            "wenc": wenc, "benc": benc, "wkvs": wkvs,
            "ident": ident, "onescol": ones,
        })
    return in_maps


def kernel(**inputs):
    global _BUILT, LAST_RESULTS
    from concourse import bass_utils

    if _BUILT is None:
        _BUILT = _build_bass()
    nc = _BUILT

    in_maps = _host_inputs(inputs)
    res = bass_utils.run_bass_kernel_spmd(
        nc, in_maps, core_ids=list(range(NCORES)), trace=TRACE)
    LAST_RESULTS = res

    others = np.array([[j for j in range(C) if j != i] for i in range(C)])
    attn_b = np.concatenate([res.results[c]["attn_o"] for c in range(NCORES)],
                            axis=0)                      # [B, C, H, D]
    logi_b = np.concatenate([res.results[c]["logi_o"] for c in range(NCORES)],
                            axis=0)                      # [B, C, C, H]
    prob_b = np.concatenate([res.results[c]["prob_o"] for c in range(NCORES)],
                            axis=0)
    attn = np.ascontiguousarray(attn_b.transpose(1, 2, 0, 3))
    idx = others[None, :, :, None]                       # [1, C, C-1, 1]
    logi = np.take_along_axis(logi_b, idx, axis=2).transpose(1, 3, 0, 2)
    prob = np.take_along_axis(prob_b, idx, axis=2).transpose(1, 3, 0, 2)
    return (attn, np.ascontiguousarray(logi), np.ascontiguousarray(prob))


# revision 16
# speedup vs baseline: 1.1062x; 1.0664x over previous
"""Trainium2 Bass kernel for nn_ClusterCritic (gnn_message_passing).

Data-parallel over 8 NeuronCores: batch axis B=32768 sharded 8x4096.
Per core, two phases:
  phase 1: stream inputs (fp32->bf16 cast DMA), PE-transpose to feature-major
           x^T tiles resident in SBUF, accumulate per-feature sum / sum-of-
           squares in PSUM via PE matmuls (batch is the contraction axis).
  barrier: AllReduce of the (sum, sumsq) stats across the 8 cores, then fold
           BatchNorm into the encoder weights (W' = W*rsqrt(var+eps),
           b' = b - m @ W'^T) on device.
  phase 2: per 512-batch chunk: fused sa/s encoder matmuls -> bias+leaky-relu
           -> fused keys/vals/sel matmul emitted directly in batch-major
           (activation as the stationary operand) -> leave-one-out attention
           (products + d/j tree-contractions on DVE, exp on ACT) -> outputs.
"""

import numpy as np
import ml_dtypes

# ---- static problem configuration (hardcoded per contract) ----
A, C, PER = 10, 5, 2
B = 32768
SD, AD = 100, 20
F = SD + AD            # 120 features per agent
HID, H, D = 32, 4, 8
ND = H * D             # 32
NCORES = 8
BC = B // NCORES       # 4096 batch per core
NCH = 8                # chunks per core
BCH = BC // NCH        # 512 batch per chunk
NSUB = 4               # 128-row subtiles per chunk
F2 = 128           # agent feature block padded for FWL
BN_EPS = 1e-5
INV_SQRT_D = float(1.0 / np.sqrt(D))

_BUILT = None  # cached Bass module
TRACE = False
LAST_RESULTS = None


def _build_bass(single=False):
    import contextlib
    import concourse.bass as bass
    import concourse.tile as tile
    from concourse import bacc, mybir

    f32 = mybir.dt.float32
    bf16 = mybir.dt.bfloat16
    AluT = mybir.AluOpType
    AF = mybir.ActivationFunctionType

    ncores = 1 if single else NCORES
    nc = bacc.Bacc("TRN2", target_bir_lowering=False, debug=False,
                   enable_asserts=False, num_devices=ncores)

    # ---------------- DRAM I/O ----------------
    states = nc.dram_tensor("states", [A, BC, SD], f32, kind="ExternalInput")
    actions = nc.dram_tensor("actions", [A, BC, AD], f32, kind="ExternalInput")
    wenc = nc.dram_tensor("wenc", [C, 2, F, 64], f32, kind="ExternalInput")
    benc = nc.dram_tensor("benc", [C, 64], f32, kind="ExternalInput")
    wkvs = nc.dram_tensor("wkvs", [65, 128], f32, kind="ExternalInput")
    ident = nc.dram_tensor("ident", [128, 128], bf16, kind="ExternalInput")
    onescol = nc.dram_tensor("onescol", [128, 1], bf16, kind="ExternalInput")

    # kernel-local output layouts (host reassembles):
    #   attn_o [b, i, n, d]; logi_o/prob_o [b, i, j, n] with the full 5x5
    #   (i, j) grid including the (masked) self column.
    attn_o = nc.dram_tensor("attn_o", [BC, C, H, D], f32, kind="ExternalOutput")
    logi_o = nc.dram_tensor("logi_o", [BC, C, C, H], f32,
                            kind="ExternalOutput")
    prob_o = nc.dram_tensor("prob_o", [BC, C, C, H], f32,
                            kind="ExternalOutput")

    with tile.TileContext(nc) as tc, contextlib.ExitStack() as ctx:
        persist = ctx.enter_context(tc.tile_pool(name="persist", bufs=1))
        dram = ctx.enter_context(tc.tile_pool(name="dram", bufs=1, space="DRAM"))
        natp = ctx.enter_context(tc.tile_pool(name="nat", bufs=2))
        sqp = ctx.enter_context(tc.tile_pool(name="sq", bufs=2))  # square scratch
        tpsum = ctx.enter_context(tc.tile_pool(name="tpsum", bufs=2, space="PSUM"))
        encps = ctx.enter_context(tc.tile_pool(name="encps", bufs=2, space="PSUM"))
        kvsps = ctx.enter_context(tc.tile_pool(name="kvsps", bufs=2, space="PSUM"))
        sasp = ctx.enter_context(tc.tile_pool(name="sas", bufs=2))
        kvbp = ctx.enter_context(tc.tile_pool(name="kvb", bufs=2))
        attp = ctx.enter_context(tc.tile_pool(name="att", bufs=2))
        smallp = ctx.enter_context(tc.tile_pool(name="small", bufs=2))

        # ---------------- persistent SBUF ----------------
        xT = persist.tile([F, A * BC], bf16)        # resident x^T per agent
        ident_sb = persist.tile([128, 128], bf16)
        ones_sb = persist.tile([128, 1], bf16)
        wenc_sb = persist.tile([F, C * 2 * 64], f32)
        wenc_bf = persist.tile([F, C * 2 * 64], bf16)
        benc_sb = persist.tile([64, C], f32)
        wkvs_sb = persist.tile([65, 128], f32)
        wkvs_bf = persist.tile([65, 128], bf16)
        stats_stage = persist.tile([F, 2 * A * NCH], f32)
        stats_loc = persist.tile([F, 2 * A], f32)
        stats_locB = persist.tile([F, 2 * A], f32)
        stats_sb = persist.tile([F, 2 * A], f32)
        m_sb = persist.tile([F, A], f32)
        mbf_sb = persist.tile([F, A], bf16)
        r_sb = persist.tile([F, A], f32)
        tmp_sb = persist.tile([F, A], f32)
        badj = persist.tile([64, C], f32)
        sas2 = [persist.tile([65, BCH], bf16, name=f"sas{i}", tag=f"sas{i}")
                for i in range(2)]

        nc.sync.dma_start(ident_sb[:], ident[:, :])
        nc.sync.dma_start(ones_sb[:], onescol[:, :])
        nc.sync.dma_start(
            wenc_sb[:].rearrange("f (c h m) -> f c h m", c=C, h=2),
            wenc.ap().rearrange("c h f m -> f c h m"))
        nc.sync.dma_start(benc_sb[:], benc.ap().rearrange("c m -> m c"))
        nc.sync.dma_start(wkvs_sb[:], wkvs[:, :])
        nc.scalar.activation(wkvs_bf[:], wkvs_sb[:], AF.Copy)
        nc.vector.memset(sas2[0][64:65, :], 1.0)
        nc.vector.memset(sas2[1][64:65, :], 1.0)


        # ---------------- phase 1: load + transpose + stats ----------------
        # stats_stage cols: sum at a*NCH+ch ; sumsq at A*NCH + a*NCH+ch
        for ch in range(NCH):
            nat = natp.tile([128, NSUB * A * F2], bf16)
            nat4 = nat[:].rearrange("p (s a f) -> p s a f", s=NSUB, a=A)
            nc.gpsimd.memset(nat4[:, :, :, F:F2], 0.0)
            for sub in range(NSUB):
                b0 = ch * BCH + sub * 128
                nc.gpsimd.dma_start(
                    nat4[:, sub, :, 0:SD],
                    states[:, b0:b0 + 128, :].rearrange("a p f -> p a f"),
                )
                nc.gpsimd.dma_start(
                    nat4[:, sub, :, SD:F],
                    actions[:, b0:b0 + 128, :].rearrange("a p f -> p a f"),
                )
            for a in range(A):
                tp = tpsum.tile([128, NSUB * 128], bf16)
                for sub in range(NSUB):
                    off = (sub * A + a) * F2
                    nc.tensor.transpose(tp[:, sub * 128:(sub + 1) * 128],
                                        nat[:, off:off + F2], ident_sb[:])
                # evacuate + batch-sum via the activation accumulator
                nc.scalar.activation(
                    xT[:, a * BC + ch * BCH: a * BC + (ch + 1) * BCH],
                    tp[0:F, :], AF.Copy,
                    accum_out=stats_stage[:, a * NCH + ch: a * NCH + ch + 1])
                sqs = sqp.tile([F, NSUB * 128], bf16, tag="sqs")
                nc.scalar.activation(
                    sqs[:], tp[0:F, :], AF.Square,
                    accum_out=stats_stage[:, A * NCH + a * NCH + ch:
                                          A * NCH + a * NCH + ch + 1])

        # ---------------- barrier: AllReduce of stats (split in two) ----------
        stg3 = stats_stage[:].rearrange("f (k a c) -> f k a c", k=2, a=A)
        loc3 = stats_loc[:].rearrange("f (a k) -> f a k", k=2)
        locB3 = stats_locB[:].rearrange("f (a k) -> f a k", k=2)
        # first half (chunks 0-3) — overlaps with phase-1 chunks 4-7
        for k in range(2):
            nc.vector.tensor_reduce(loc3[:, :, k], stg3[:, k, :, 0:NCH // 2],
                                    mybir.AxisListType.X, AluT.add)
        cc_in = dram.tile([F, 2 * A], f32)
        cc_out = dram.tile([F, 2 * A], f32)
        cc_inB = dram.tile([F, 2 * A], f32, tag="ccb")
        cc_outB = dram.tile([F, 2 * A], f32, tag="ccb2")
        nc.sync.dma_start(cc_in[:], stats_loc[:])
        if single:
            nc.sync.dma_start(cc_out[:], cc_in[:])
        else:
            nc.gpsimd.collective_compute(
                "AllReduce", AluT.add,
                replica_groups=[list(range(NCORES))],
                ins=[cc_in[:].opt()], outs=[cc_out[:].opt()],
            )
        # second half (chunks 4-7)
        for k in range(2):
            nc.vector.tensor_reduce(locB3[:, :, k], stg3[:, k, :, NCH // 2:],
                                    mybir.AxisListType.X, AluT.add)
        nc.sync.dma_start(cc_inB[:], stats_locB[:])
        if single:
            nc.sync.dma_start(cc_outB[:], cc_inB[:])
        else:
            nc.gpsimd.collective_compute(
                "AllReduce", AluT.add,
                replica_groups=[list(range(NCORES))],
                ins=[cc_inB[:].opt()], outs=[cc_outB[:].opt()],
            )
        nc.sync.dma_start(stats_sb[:], cc_out[:])
        nc.sync.dma_start(stats_locB[:], cc_outB[:])
        nc.vector.tensor_tensor(stats_sb[:], stats_sb[:], stats_locB[:],
                                AluT.add)

        # m = S1/B ; var = S2/B - m^2 ; r = 1/sqrt(var+eps)
        st3 = stats_sb[:].rearrange("f (a two) -> f a two", two=2)
        nc.vector.tensor_scalar(m_sb[:], st3[:, :, 0], 1.0 / B, None, AluT.mult)
        nc.vector.tensor_tensor(tmp_sb[:], m_sb[:], m_sb[:], AluT.mult)
        nc.vector.tensor_scalar(r_sb[:], st3[:, :, 1], 1.0 / B, None, AluT.mult)
        nc.vector.tensor_tensor(r_sb[:], r_sb[:], tmp_sb[:], AluT.subtract)
        nc.vector.tensor_scalar(r_sb[:], r_sb[:], BN_EPS, None, AluT.add)
        nc.scalar.activation(tmp_sb[:], r_sb[:], AF.Sqrt)
        nc.vector.reciprocal(r_sb[:], tmp_sb[:])
        nc.vector.tensor_copy(mbf_sb[:], m_sb[:])

        # W' = W * r (per-feature row scale), cast to bf16
        for c in range(C):
            for h2 in range(2):
                a = 2 * c + h2
                sl = slice((c * 2 + h2) * 64, (c * 2 + h2) * 64 + 64)
                nc.scalar.activation(wenc_bf[:, sl], wenc_sb[:, sl], AF.Copy,
                                     scale=r_sb[:, a:a + 1])
        # badj[c] = benc[c] - m^T @ W'
        for c in range(C):
            bps = encps.tile([64, BCH], f32, tag="eps")
            for h2 in range(2):
                a = 2 * c + h2
                sl = slice((c * 2 + h2) * 64, (c * 2 + h2) * 64 + 64)
                nc.tensor.matmul(bps[:, 0:1], wenc_bf[:, sl],
                                 mbf_sb[:, a:a + 1],
                                 start=(h2 == 0), stop=(h2 == 1))
            nc.scalar.activation(badj[:, c:c + 1], bps[:, 0:1], AF.Identity,
                                 bias=benc_sb[:, c:c + 1], scale=-1.0)

        # ---------------- phase 2: encoder + attention ----------------
        for ch in range(NCH):
            # KVSb: [128b, (sub, c, t)]  t: k 0-31 | v 32-63 | v001 64-95 |
            # sel 96-127
            kvsb = kvbp.tile([128, NSUB * C * 128], bf16)
            kv4 = kvsb[:].rearrange("p (s c t) -> p s c t", s=NSUB, c=C)
            for c in range(C):
                eps_t = encps.tile([64, BCH], f32, tag="eps")
                for h2 in range(2):
                    a = 2 * c + h2
                    sl = slice((c * 2 + h2) * 64, (c * 2 + h2) * 64 + 64)
                    nc.tensor.matmul(
                        eps_t[:], wenc_bf[:, sl],
                        xT[:, a * BC + ch * BCH: a * BC + (ch + 1) * BCH],
                        start=(h2 == 0), stop=(h2 == 1))
                sas = sas2[c % 2]
                nc.scalar.activation(sas[0:64, :], eps_t[:], AF.Identity,
                                     bias=badj[:, c:c + 1])
                z = sasp.tile([64, BCH], bf16, tag="zt")
                nc.vector.tensor_scalar(z[:], sas[0:64, :], 0.01, None,
                                        AluT.mult)
                nc.vector.tensor_tensor(sas[0:64, :], sas[0:64, :], z[:],
                                        AluT.max)
                # kvs in batch-major: out[128b, 128t] = sas_sub^T @ wkvs
                kps = kvsps.tile([128, NSUB * 128], f32)
                for sub in range(NSUB):
                    nc.tensor.matmul(
                        kps[:, sub * 128:(sub + 1) * 128],
                        sas[:, sub * 128:(sub + 1) * 128], wkvs_bf[:],
                        start=(sub == 0), stop=(sub == NSUB - 1))
                nc.scalar.activation(
                    kv4[:, :, c, :],
                    kps[:].rearrange("p (s t) -> p s t", s=NSUB), AF.Copy)

            # vals leaky-relu: v = max(v+b, 0.01*(v+b)) via the v001 columns
            nc.vector.tensor_tensor(kv4[:, :, :, 32:64], kv4[:, :, :, 32:64],
                                    kv4[:, :, :, 64:96], AluT.max)

            # logits products: lp[128, (sub, i, j, nd)] = sel_i * k_j
            lp = attp.tile([128, NSUB * C * C * ND], bf16, tag="lp")
            lp5 = lp[:].rearrange("p (s i j t) -> p s i j t", s=NSUB, i=C, j=C)
            for sub in range(NSUB):
                sel_v = (kv4[:, sub, :, 96:128]
                         .rearrange("p i (one t) -> p i one t", one=1)
                         .broadcast_to([128, C, C, ND]))
                key_v = (kv4[:, sub, :, 0:32]
                         .rearrange("p (one j) t -> p one j t", one=1)
                         .broadcast_to([128, C, C, ND]))
                nc.vector.tensor_tensor(lp5[:, sub], sel_v, key_v, AluT.mult)
            # d-contraction 8 -> 1 by halving adds; layout (q=(s,i,j), n, d)
            lpn = lp[:].rearrange("p (q n d) -> p q n d", n=H, d=D)
            l1 = attp.tile([128, NSUB * C * C * H * 4], bf16, tag="l1")
            l1v = l1[:].rearrange("p (q n d) -> p q n d", n=H, d=4)
            nc.vector.tensor_tensor(l1v, lpn[:, :, :, 0:4], lpn[:, :, :, 4:8],
                                    AluT.add)
            l2 = attp.tile([128, NSUB * C * C * H * 2], bf16, tag="l2")
            l2v = l2[:].rearrange("p (q n d) -> p q n d", n=H, d=2)
            nc.vector.tensor_tensor(l2v, l1v[:, :, :, 0:2], l1v[:, :, :, 2:4],
                                    AluT.add)
            lgt = attp.tile([128, NSUB * C * C * H], f32, tag="lgt")
            lgtv = lgt[:].rearrange("p (q n d) -> p q n d", n=H, d=1)
            nc.vector.tensor_tensor(lgtv, l2v[:, :, :, 0:1], l2v[:, :, :, 1:2],
                                    AluT.add)

            # self mask: slots k = i*(C+1) in the (i,j) grid
            mask_v = (lgt[:].rearrange("p (s k n) -> p s k n", s=NSUB, k=C * C)
                      [:, :, 0:C * C:C + 1, :])
            nc.vector.memset(mask_v, -1e30)

            # exp((logits - 0)/sqrt(D)) -> bf16 (no max-subtraction needed:
            # |logits| is O(1) for this model)
            expt = attp.tile([128, NSUB * C * C * H], bf16, tag="expt")
            nc.scalar.activation(expt[:], lgt[:], AF.Exp, scale=INV_SQRT_D)

            ex5 = expt[:].rearrange("p (s i j n) -> p s i j n", s=NSUB, i=C,
                                    j=C)
            exq = expt[:].rearrange("p (q j n) -> p q j n", j=C, n=H)
            # denominator: sum over j (5 = 2+2+1)
            r1 = smallp.tile([128, NSUB * C * 2 * H], bf16, tag="r1")
            r1v = r1[:].rearrange("p (q j n) -> p q j n", j=2, n=H)
            nc.vector.tensor_tensor(r1v, exq[:, :, 0:2, :], exq[:, :, 2:4, :],
                                    AluT.add)
            rs = smallp.tile([128, NSUB * C * H], f32, tag="rs")
            rsv = rs[:].rearrange("p (q one n) -> p q one n", one=1, n=H)
            nc.vector.tensor_tensor(rsv, r1v[:, :, 0:1, :], r1v[:, :, 1:2, :],
                                    AluT.add)
            nc.vector.tensor_tensor(rsv, rsv, exq[:, :, 4:5, :], AluT.add)
            rc = smallp.tile([128, NSUB * C * H], f32, tag="rc")
            nc.vector.reciprocal(rc[:], rs[:])

            # probs (fp32 output): expt * rc broadcast over j
            probs = attp.tile([128, NSUB * C * C * H], f32, tag="probs")
            rcb = (rc[:].rearrange("p (q n) -> p q n", n=H)
                   .rearrange("p q (one n) -> p q one n", one=1)
                   .broadcast_to([128, NSUB * C, C, H]))
            nc.vector.tensor_tensor(
                probs[:].rearrange("p (q j n) -> p q j n", j=C, n=H),
                exq, rcb, AluT.mult)

            # attention products: at[(sub,i,j,(n,d))] = expt(i,j,n) * v(j,n,d)
            at = attp.tile([128, NSUB * C * C * ND], bf16, tag="at")
            at5 = at[:].rearrange("p (s i j n d) -> p s i j n d", s=NSUB, i=C,
                                  j=C, n=H)
            for sub in range(NSUB):
                e_v = (ex5[:, sub]
                       .rearrange("p i j (n one) -> p i j n one", one=1)
                       .broadcast_to([128, C, C, H, D]))
                v_v = (kv4[:, sub, :, 32:64]
                       .rearrange("p (one j) t -> p one j t", one=1)
                       .broadcast_to([128, C, C, ND])
                       .rearrange("p i j (n d) -> p i j n d", n=H))
                nc.vector.tensor_tensor(at5[:, sub], e_v, v_v, AluT.mult)
            # j-contraction (5 = 2+2+1)
            atq = at[:].rearrange("p (q j t) -> p q j t", j=C, t=ND)
            a1 = attp.tile([128, NSUB * C * 2 * ND], bf16, tag="a1")
            a1v = a1[:].rearrange("p (q j t) -> p q j t", j=2, t=ND)
            nc.vector.tensor_tensor(a1v, atq[:, :, 0:2, :], atq[:, :, 2:4, :],
                                    AluT.add)
            au = attp.tile([128, NSUB * C * ND], bf16, tag="au")
            auv = au[:].rearrange("p (q one t) -> p q one t", one=1, t=ND)
            nc.vector.tensor_tensor(auv, a1v[:, :, 0:1, :], a1v[:, :, 1:2, :],
                                    AluT.add)
            nc.vector.tensor_tensor(auv, auv, atq[:, :, 4:5, :], AluT.add)
            # final scale by rc broadcast over d -> fp32
            attn = attp.tile([128, NSUB * C * ND], f32, tag="attn")
            rcd = (rc[:].rearrange("p (q n) -> p q n", n=H)
                   .rearrange("p q (n one) -> p q n one", one=1)
                   .broadcast_to([128, NSUB * C, H, D]))
            nc.vector.tensor_tensor(
                attn[:].rearrange("p (q n d) -> p q n d", n=H, d=D),
                au[:].rearrange("p (q n d) -> p q n d", n=H, d=D), rcd,
                AluT.mult)

            # ---------------- output DMAs (HWDGE) ----------------
            nc.sync.dma_start(
                attn_o[ch * BCH:(ch + 1) * BCH]
                .rearrange("(s p) i n d -> p s (i n d)", p=128),
                attn[:].rearrange("p (s x) -> p s x", s=NSUB),
            )
            nc.sync.dma_start(
                logi_o[ch * BCH:(ch + 1) * BCH]
                .rearrange("(s p) i j n -> p s (i j n)", p=128),
                lgt[:].rearrange("p (s x) -> p s x", s=NSUB),
            )
            nc.sync.dma_start(
                prob_o[ch * BCH:(ch + 1) * BCH]
                .rearrange("(s p) i j n -> p s (i j n)", p=128),
                probs[:].rearrange("p (s x) -> p s x", s=NSUB),
            )
    nc.compile()
    return nc


def _host_inputs(inputs):
    """Slice batch per core + prepack weights into device layouts."""
    states = np.asarray(inputs["states"], np.float32)
    actions = np.asarray(inputs["actions"], np.float32)
    enc_W = np.asarray(inputs["enc_W"], np.float32)     # [C, HID, 240]
    enc_b = np.asarray(inputs["enc_b"], np.float32)     # [C, HID]
    senc_W = np.asarray(inputs["senc_W"], np.float32)   # [C, HID, 200]
    senc_b = np.asarray(inputs["senc_b"], np.float32)   # [C, HID]
    key_W = np.asarray(inputs["key_W"], np.float32)     # [H, D, HID]
    sel_W = np.asarray(inputs["sel_W"], np.float32)
    val_W = np.asarray(inputs["val_W"], np.float32)
    val_b = np.asarray(inputs["val_b"], np.float32)     # [H, D]

    wenc = np.zeros((C, 2, F, 64), np.float32)
    for c in range(C):
        for h2 in range(2):
            wsa = np.concatenate(
                [enc_W[c, :, h2 * SD:(h2 + 1) * SD],
                 enc_W[c, :, 2 * SD + h2 * AD: 2 * SD + (h2 + 1) * AD]],
                axis=1)                                   # [32, 120]
            ws = senc_W[c, :, h2 * SD:(h2 + 1) * SD]      # [32, 100]
            wenc[c, h2, :, 0:32] = wsa.T
            wenc[c, h2, 0:SD, 32:64] = ws.T
    benc = np.ascontiguousarray(
        np.concatenate([enc_b, senc_b], axis=1))          # [C, 64]

    wkvs = np.zeros((65, 128), np.float32)
    kw = key_W.reshape(ND, HID).T
    vw = val_W.reshape(ND, HID).T
    sw = sel_W.reshape(ND, HID).T
    vb = val_b.reshape(ND)
    wkvs[0:32, 0:32] = kw
    wkvs[0:32, 32:64] = vw
    wkvs[0:32, 64:96] = 0.01 * vw
    wkvs[32:64, 96:128] = sw
    wkvs[64, 32:64] = vb
    wkvs[64, 64:96] = 0.01 * vb

    ident = np.eye(128, dtype=ml_dtypes.bfloat16)
    ones = np.ones((128, 1), ml_dtypes.bfloat16)

    in_maps = []
    for core in range(NCORES):
        sl = slice(core * BC, (core + 1) * BC)
        in_maps.append({
            "states": np.ascontiguousarray(states[:, sl, :]),
            "actions": np.ascontiguousarray(actions[:, sl, :]),
            "wenc": wenc, "benc": benc, "wkvs": wkvs,
            "ident": ident, "onescol": ones,
        })
    return in_maps


def kernel(**inputs):
    global _BUILT, LAST_RESULTS
    from concourse import bass_utils

    if _BUILT is None:
        _BUILT = _build_bass()
    nc = _BUILT

    in_maps = _host_inputs(inputs)
    res = bass_utils.run_bass_kernel_spmd(
        nc, in_maps, core_ids=list(range(NCORES)), trace=TRACE)
    LAST_RESULTS = res

    others = np.array([[j for j in range(C) if j != i] for i in range(C)])
    attn_b = np.concatenate([res.results[c]["attn_o"] for c in range(NCORES)],
                            axis=0)                      # [B, C, H, D]
    logi_b = np.concatenate([res.results[c]["logi_o"] for c in range(NCORES)],
                            axis=0)                      # [B, C, C, H]
    prob_b = np.concatenate([res.results[c]["prob_o"] for c in range(NCORES)],
                            axis=0)
    attn = np.ascontiguousarray(attn_b.transpose(1, 2, 0, 3))
    idx = others[None, :, :, None]                       # [1, C, C-1, 1]
    logi = np.take_along_axis(logi_b, idx, axis=2).transpose(1, 3, 0, 2)
    prob = np.take_along_axis(prob_b, idx, axis=2).transpose(1, 3, 0, 2)
    return (attn, np.ascontiguousarray(logi), np.ascontiguousarray(prob))
